# revision 45
# baseline (speedup 1.0000x reference)
"""AttentiveFP readout kernel for 8 Trainium2 NeuronCores (v4).

Graph-contiguous sharding of V=500k nodes across 8 cores (seg_ids
sorted; split at graph boundaries). All segment ops core-local, no
collectives.

Engine plan (v1 was DVE-bound at 70%; v2/v3 fixed op selection):
- node features staged by HOST as bf16 in four device layouts:
  nfaug  [128p, NT, NSUB, 257]  (col0 = valid flag, cols 1.. = nf)
  nft    [128f, NT, 2, NSUB*128] (transposed, for w01 on PE)
  mn     [128p, NT, 128, NSUB]  one-hot node->graph, g-major
  mnt    [128g, NT, NSUB, 128]  its transpose (u-gather on PE)
- per-node logits w01 = nf . wln_t : PE matmuls, nfT stationary, N=2.
- u broadcast/gather to nodes: PE matmuls MnT_s stationary @ ucol, N=1.
- e = exp(lrelu(z)) via sigmoid identity (exp table never loaded ->
  single resident ACT table set, no ACT_TABLE_LOAD thrash):
    sm = sig(-z/4); q = (sm-1)/sm = -e^{z/4}; e = (q^2)^2
- attention weights folded into the ONE-HOT (not nf): Mne = Mn * e with
  one packed tensor_tensor per timestep (innermost NSUB axis keeps the
  2x DVE mode; per-partition AP-scalar tensor_scalar is a hw slow path).
- elu(x) = relu(x) + min(e^x, 1) - 1, e^x via sigmoid ratio (x<=0 so
  no cancellation); the -1 folded into the GRU input bias on host.
- GRU/Wp biases folded into K=1 ones-row matmuls.
- v4: tiles processed in PAIRS with stage-interleaved emission so one
  tile's PE matmuls fill the other tile's cross-engine dependency gaps
  (engines execute their queues in issue order).
"""

import numpy as np
from contextlib import ExitStack

import concourse.bass as bass
import concourse.bacc as bacc
import concourse.mybir as mybir
from concourse import tile
from concourse.bass_utils import run_bass_kernel_spmd

F32 = mybir.dt.float32
BF16 = mybir.dt.bfloat16
NP_BF16 = mybir.dt.np(mybir.dt.bfloat16)
AOP = mybir.AluOpType
ACT = mybir.ActivationFunctionType
AX = mybir.AxisListType

NCORES = 8
F = 256
T = 2
LAST_RESULT = None


def _build_program(NT, NSUB, bl_vals):
    ctx = ExitStack()
    nc = bacc.Bacc("TRN2")
    nc.all_engine_barrier()

    NN = NSUB * 128  # node slots per tile

    nfaug_d = nc.dram_tensor("nfaug", [128, NT, NSUB, F + 1], BF16, kind="ExternalInput")
    nft_d = nc.dram_tensor("nft", [128, NT, 2, NN], BF16, kind="ExternalInput")
    mn_d = nc.dram_tensor("mn", [128, NT, 128, NSUB], BF16, kind="ExternalInput")
    mnt_d = nc.dram_tensor("mnt", [128, NT, NSUB, 128], BF16, kind="ExternalInput")
    identb_d = nc.dram_tensor("identb", [128, 128], BF16, kind="ExternalInput")
    onesrow_d = nc.dram_tensor("onesrow", [1, 128], BF16, kind="ExternalInput")
    wl2_d = nc.dram_tensor("wl2", [128, 2, T], BF16, kind="ExternalInput")
    wlnf_d = [nc.dram_tensor(f"wlnf{t}", [128, F], BF16, kind="ExternalInput") for t in range(T)]
    wlg_d = [nc.dram_tensor(f"wlg{t}", [128, F], BF16, kind="ExternalInput") for t in range(T)]
    wpt_d = [nc.dram_tensor(f"wpt{t}", [128, 2, F], BF16, kind="ExternalInput") for t in range(T)]
    wih_d = [nc.dram_tensor(f"wih{t}", [128, 2, 3 * F], BF16, kind="ExternalInput") for t in range(T)]
    whh_d = [nc.dram_tensor(f"whh{t}", [128, 2, 3 * F], BF16, kind="ExternalInput") for t in range(T)]
    bprow_d = [nc.dram_tensor(f"bprow{t}", [1, F], BF16, kind="ExternalInput") for t in range(T)]
    brzrow_d = [nc.dram_tensor(f"brzrow{t}", [1, 2 * F], BF16, kind="ExternalInput") for t in range(T)]
    binrow_d = [nc.dram_tensor(f"binrow{t}", [1, F], BF16, kind="ExternalInput") for t in range(T)]
    bhnrow_d = [nc.dram_tensor(f"bhnrow{t}", [1, F], BF16, kind="ExternalInput") for t in range(T)]
    out_d = nc.dram_tensor("out", [NT * 128, F], F32, kind="ExternalOutput")

    with tile.TileContext(nc) as tc:
      with tc.sbuf_pool(name="const", bufs=1) as cpool, \
           tc.sbuf_pool(name="work", bufs=4) as wpool, \
           tc.sbuf_pool(name="scr", bufs=3) as scrpool, \
           tc.sbuf_pool(name="small", bufs=3) as spool, \
           tc.psum_pool(name="pacc", bufs=4) as pacc, \
           tc.psum_pool(name="prz", bufs=1) as prz, \
           tc.psum_pool(name="ptiny", bufs=3) as ptiny:

        identb = cpool.tile_from(identb_d[:, :], name="identb")
        onesrow = cpool.tile_from(onesrow_d[:, :], name="onesrow")
        wl2 = cpool.tile_from(wl2_d[:, :, :], name="wl2")
        wlnf = [cpool.tile_from(wlnf_d[t][:, :], name=f"wlnf{t}") for t in range(T)]
        wlg = [cpool.tile_from(wlg_d[t][:, :], name=f"wlg{t}") for t in range(T)]
        wpt = [cpool.tile_from(wpt_d[t][:, :, :], name=f"wpt{t}") for t in range(T)]
        wih = [cpool.tile_from(wih_d[t][:, :, :], name=f"wih{t}") for t in range(T)]
        whh = [cpool.tile_from(whh_d[t][:, :, :], name=f"whh{t}") for t in range(T)]
        bprow = [cpool.tile_from(bprow_d[t][:, :], name=f"bprow{t}") for t in range(T)]
        brzrow = [cpool.tile_from(brzrow_d[t][:, :], name=f"brzrow{t}") for t in range(T)]
        binrow = [cpool.tile_from(binrow_d[t][:, :], name=f"binrow{t}") for t in range(T)]
        bhnrow = [cpool.tile_from(bhnrow_d[t][:, :], name=f"bhnrow{t}") for t in range(T)]

        S = {}  # per-tile live state

        def emit_dma(j):
            s = S[j] = {}
            s["nfaug"] = wpool.tile([128, NSUB, F + 1], BF16, name=f"nfaug{j}", tag="nfaug")
            s["nft"] = wpool.tile([128, 2, NN], BF16, name=f"nft{j}", tag="nft", bufs=2)
            s["mn"] = wpool.tile([128, 128, NSUB], BF16, name=f"mn{j}", tag="mn")
            s["mnt"] = wpool.tile([128, NSUB, 128], BF16, name=f"mnt{j}", tag="mnt")
            nc.sync.dma_start(s["nfaug"][:, :, :], nfaug_d[:, j, :, :])
            nc.scalar.dma_start(s["nft"][:, :, :], nft_d[:, j, :, :])
            nc.gpsimd.dma_start(s["mn"][:, :, :], mn_d[:, j, :, :])
            nc.sync.dma_start(s["mnt"][:, :, :], mnt_d[:, j, :, :])

        def emit_g0(j):
            s = S[j]
            ps_g0 = pacc.tile([128, F], F32, name=f"psg0_{j}", tag="acc")
            for k in range(NSUB):
                nc.tensor.matmul(ps_g0[:, :], s["mn"][:, :, k],
                                 s["nfaug"][:, k, 1:F + 1],
                                 start=(k == 0), stop=(k == NSUB - 1))
            gf = spool.tile([128, F], BF16, name=f"gf0_{j}", tag="gf", bufs=8)
            nc.scalar.copy(gf[:, :], ps_g0[:, :])
            s["gf"] = gf

        H = NSUB  # w01 subtiles on PE (small matmuls pipeline ~free there)

        def emit_w01(j):
            s = S[j]
            w01f = spool.tile([128, T, NSUB], F32, name=f"w01_{j}", tag="w01")
            ps_w01 = ptiny.tile([128, H, T], F32, name=f"psw01_{j}", tag="tiny")
            for k in range(H):
                for c in range(2):
                    nc.tensor.matmul(ps_w01[:, k, :],
                                     s["nft"][:, c, k * 128:(k + 1) * 128],
                                     wl2[:, c, :],
                                     start=(c == 0), stop=(c == 1))
            nc.vector.tensor_copy(w01f[:, :, 0:H],
                                  ps_w01[:, :, :].transpose((0, 2, 1)))
            for k in range(H, NSUB):
                for t in range(T):
                    scr = spool.tile([128, F], BF16, name=f"w1s_{j}_{k}_{t}", tag="w1s")
                    nc.vector.scalar_tensor_tensor(
                        scr[:, :], s["nfaug"][:, k, 1:F + 1], 1.0,
                        wlnf[t][:, :], op0=AOP.mult, op1=AOP.mult,
                        accum_out=w01f[:, t, k:k + 1])
            s["w01"] = w01f

        def emit_u(j, t):
            s = S[j]
            uscr = spool.tile([128, F], BF16, name=f"uscr_{j}_{t}", tag="uscr")
            ucol = spool.tile([128, 1], F32, name=f"ucol_{j}_{t}", tag="ucol")
            nc.vector.scalar_tensor_tensor(
                uscr[:, :], s["gf"][:, :], 0.0, wlg[t][:, :],
                op0=AOP.max, op1=AOP.mult, accum_out=ucol[:, :])
            ucolb = spool.tile([128, 1], BF16, name=f"ucolb_{j}_{t}", tag="ucolb")
            nc.scalar.activation(ucolb[:, :], ucol[:, :], ACT.Copy,
                                 bias=float(bl_vals[t]))
            s["ucolb"] = ucolb

        def emit_ubcv(j, t):
            s = S[j]
            ps_ubcv = ptiny.tile([128, NSUB], F32, name=f"psub_{j}_{t}", tag="tiny")
            for k in range(NSUB):
                nc.tensor.matmul(ps_ubcv[:, k:k + 1], s["mnt"][:, k, :],
                                 s["ucolb"][:, :], start=True, stop=True)
            s["ps_ubcv"] = ps_ubcv

        def emit_echain(j, t):
            s = S[j]
            zt = spool.tile([128, NSUB], F32, name=f"zt_{j}_{t}", tag="zt")
            nc.vector.tensor_tensor(zt[:, :], s["ps_ubcv"][:, :],
                                    s["w01"][:, t, :], op=AOP.add)
            zl = spool.tile([128, NSUB], F32, name=f"zl_{j}_{t}", tag="zl")
            nc.scalar.activation(zl[:, :], zt[:, :], ACT.Prelu, alpha=0.01)
            sm = spool.tile([128, NSUB], F32, name=f"sm_{j}_{t}", tag="sm")
            nc.scalar.activation(sm[:, :], zl[:, :], ACT.Sigmoid, scale=-0.25)
            rsm = spool.tile([128, NSUB], F32, name=f"rsm_{j}_{t}", tag="rsm")
            nc.vector.reciprocal(rsm[:, :], sm[:, :])
            q4 = spool.tile([128, NSUB], F32, name=f"q4_{j}_{t}", tag="q4")
            nc.vector.tensor_scalar(q4[:, :], rsm[:, :], -1.0, 1.0,
                                    op0=AOP.mult, op1=AOP.add)
            q2 = spool.tile([128, NSUB], F32, name=f"q2_{j}_{t}", tag="q2")
            nc.vector.tensor_tensor(q2[:, :], q4[:, :], q4[:, :], op=AOP.mult)
            ebf = spool.tile([128, NSUB], BF16, name=f"ebf_{j}_{t}", tag="ebf")
            nc.vector.tensor_tensor(ebf[:, :], q2[:, :], q2[:, :], op=AOP.mult)
            mne = scrpool.tile([128, 128, NSUB], BF16, name=f"mne_{j}_{t}", tag="mne")
            # split along subtiles so the first ds matmuls can start while
            # the second half is still scaling
            SH = NSUB // 2
            nc.vector.tensor_tensor(
                mne[:, :, 0:SH], s["mn"][:, :, 0:SH],
                ebf[:, 0:SH].unsqueeze(1).broadcast_to((128, 128, SH)),
                op=AOP.mult)
            nc.vector.tensor_tensor(
                mne[:, :, SH:NSUB], s["mn"][:, :, SH:NSUB],
                ebf[:, SH:NSUB].unsqueeze(1).broadcast_to((128, 128, NSUB - SH)),
                op=AOP.mult)
            s["mne"] = mne

        def emit_ds(j, t):
            s = S[j]
            ps_ds = pacc.tile([128, F + 1], F32, name=f"psds_{j}_{t}", tag="acc")
            for k in range(NSUB):
                nc.tensor.matmul(ps_ds[:, :], s["mne"][:, :, k],
                                 s["nfaug"][:, k, :],
                                 start=(k == 0), stop=(k == NSUB - 1))
            s["ps_ds"] = ps_ds

        def emit_stl(j, t):
            s = S[j]
            dplus = spool.tile([128, 1], F32, name=f"dp_{j}_{t}", tag="dp")
            nc.vector.tensor_scalar(dplus[:, :], s["ps_ds"][:, 0:1], 1e-30, None,
                                    op0=AOP.max)
            recd = spool.tile([128, 1], F32, name=f"recd_{j}_{t}", tag="recd")
            nc.vector.reciprocal(recd[:, :], dplus[:, :])
            stl = spool.tile([128, F], BF16, name=f"stl_{j}_{t}", tag="stl")
            nc.scalar.activation(stl[:, :], s["ps_ds"][:, 1:F + 1], ACT.Copy,
                                 scale=recd[:, :])
            s["stl"] = stl

        def emit_ctx(j, t):
            s = S[j]
            ps_st = ptiny.tile([128, 2, 128], BF16, name=f"psst_{j}_{t}", tag="tiny")
            for c in range(2):
                nc.tensor.transpose(ps_st[:, c, :],
                                    s["stl"][:, c * 128:(c + 1) * 128],
                                    identb[:, :])
            stT = spool.tile([128, 2, 128], BF16, name=f"stT_{j}_{t}", tag="stT")
            nc.vector.tensor_copy(stT[:, :, :], ps_st[:, :, :])
            ps_ctx = pacc.tile([128, F], F32, name=f"psctx_{j}_{t}", tag="acc")
            nc.tensor.matmul(ps_ctx[:, :], stT[:, 0, :], wpt[t][:, 0, :],
                             start=True, stop=False)
            nc.tensor.matmul(ps_ctx[:, :], stT[:, 1, :], wpt[t][:, 1, :],
                             start=False, stop=False)
            nc.tensor.matmul(ps_ctx[:, :], onesrow[:, :], bprow[t][:, :],
                             start=False, stop=True)
            sg = spool.tile([128, F], BF16, name=f"sg_{j}_{t}", tag="sg")
            nc.scalar.activation(sg[:, :], ps_ctx[:, :], ACT.Sigmoid)
            omy = spool.tile([128, F], F32, name=f"omy_{j}_{t}", tag="omy")
            nc.scalar.activation(omy[:, :], sg[:, :], ACT.Copy,
                                 scale=-1.0, bias=1.0)
            romy = spool.tile([128, F], F32, name=f"romy_{j}_{t}", tag="romy")
            nc.vector.reciprocal_approx_fast(out=romy[:, :], in_=omy[:, :])
            exn = spool.tile([128, F], BF16, name=f"exn_{j}_{t}", tag="exn")
            nc.vector.tensor_tensor(exn[:, :], sg[:, :], romy[:, :], op=AOP.mult)
            exm = spool.tile([128, F], BF16, name=f"exm_{j}_{t}", tag="exm")
            nc.vector.tensor_scalar(exm[:, :], exn[:, :], 1.0, None, op0=AOP.min)
            ctxb = spool.tile([128, F], BF16, name=f"ctxb_{j}_{t}", tag="ctxb")
            nc.vector.scalar_tensor_tensor(
                ctxb[:, :], ps_ctx[:, :], 0.0, exm[:, :],
                op0=AOP.max, op1=AOP.add)
            s["ctxb"] = ctxb

        def emit_gru(j, t):
            s = S[j]
            gf = s["gf"]
            ps_tr = ptiny.tile([128, 4, 128], BF16, name=f"pstr_{j}_{t}", tag="tiny")
            for c in range(2):
                nc.tensor.transpose(ps_tr[:, c, :],
                                    s["ctxb"][:, c * 128:(c + 1) * 128],
                                    identb[:, :])
                nc.tensor.transpose(ps_tr[:, 2 + c, :],
                                    gf[:, c * 128:(c + 1) * 128],
                                    identb[:, :])
            xh = spool.tile([128, 4, 128], BF16, name=f"xh_{j}_{t}", tag="xh")
            nc.scalar.copy(xh[:, :, :], ps_tr[:, :, :])
            # same-stationary matmuls adjacent (xh chunk reused for rz + n)
            ps_rz = prz.tile([128, 2 * F], F32, name=f"psrz_{j}_{t}", tag="rz")
            ps_in = pacc.tile([128, F], F32, name=f"psin_{j}_{t}", tag="acc")
            ps_hn = pacc.tile([128, F], F32, name=f"pshn_{j}_{t}", tag="acc")
            for c in range(2):
                nc.tensor.matmul(ps_rz[:, :], xh[:, c, :], wih[t][:, c, 0:2 * F],
                                 start=(c == 0), stop=False)
                nc.tensor.matmul(ps_in[:, :], xh[:, c, :],
                                 wih[t][:, c, 2 * F:3 * F],
                                 start=(c == 0), stop=False)
            for c in range(2):
                nc.tensor.matmul(ps_rz[:, :], xh[:, 2 + c, :],
                                 whh[t][:, c, 0:2 * F],
                                 start=False, stop=False)
                nc.tensor.matmul(ps_hn[:, :], xh[:, 2 + c, :],
                                 whh[t][:, c, 2 * F:3 * F],
                                 start=(c == 0), stop=False)
            nc.tensor.matmul(ps_rz[:, :], onesrow[:, :], brzrow[t][:, :],
                             start=False, stop=True)
            nc.tensor.matmul(ps_in[:, :], onesrow[:, :], binrow[t][:, :],
                             start=False, stop=True)
            nc.tensor.matmul(ps_hn[:, :], onesrow[:, :], bhnrow[t][:, :],
                             start=False, stop=True)
            rza = spool.tile([128, 2 * F], BF16, name=f"rza_{j}_{t}", tag="rza")
            nc.scalar.activation(rza[:, :], ps_rz[:, :], ACT.Sigmoid)
            tmp = spool.tile([128, F], BF16, name=f"tmp_{j}_{t}", tag="tmp")
            nc.vector.tensor_tensor(tmp[:, :], ps_hn[:, :], rza[:, 0:F],
                                    op=AOP.mult)
            t2 = spool.tile([128, F], F32, name=f"t2_{j}_{t}", tag="t2")
            nc.vector.tensor_tensor(t2[:, :], ps_in[:, :], tmp[:, :], op=AOP.add)
            nn = spool.tile([128, F], BF16, name=f"nn_{j}_{t}", tag="nn")
            nc.scalar.activation(nn[:, :], t2[:, :], ACT.Tanh)
            hm = spool.tile([128, F], BF16, name=f"hm_{j}_{t}", tag="hm")
            nc.vector.tensor_tensor(hm[:, :], gf[:, :], nn[:, :], op=AOP.subtract)
            hz = spool.tile([128, F], BF16, name=f"hz_{j}_{t}", tag="hz")
            nc.vector.tensor_tensor(hz[:, :], hm[:, :], rza[:, F:2 * F],
                                    op=AOP.mult)
            if t < T - 1:
                gf_new = spool.tile([128, F], BF16, name=f"gfn_{j}_{t}", tag="gf", bufs=8)
                nc.vector.tensor_tensor(gf_new[:, :], hz[:, :], nn[:, :],
                                        op=AOP.add)
                s["gf"] = gf_new
                # hoist next timestep's u so its DVE/ACT chain overlaps the
                # partner tile's GRU instead of stalling the next seam
                emit_u(j, t + 1)
            else:
                gout = spool.tile([128, F], F32, name=f"gout_{j}", tag="gout")
                nc.vector.tensor_tensor(gout[:, :], hz[:, :], nn[:, :],
                                        op=AOP.add)
                nc.scalar.dma_start(out_d[j * 128:(j + 1) * 128, :], gout[:, :])
                del S[j]

        pairs = [tuple(j for j in (j0, j0 + 1) if j < NT)
                 for j0 in range(0, NT, 2)]
        # software pipeline: pair P+1's DMA issues at the start of pair P;
        # pair P+1's g0/w01 matmuls are emitted between P's two timesteps
        # to fill the PE bubble at the t0->t1 dependency chain.
        for j in pairs[0]:
            emit_dma(j)
        for j in pairs[0]:
            emit_g0(j)
        for j in pairs[0]:
            emit_w01(j)
        for p, pair in enumerate(pairs):
            nxt = pairs[p + 1] if p + 1 < len(pairs) else ()
            for j in nxt:
                emit_dma(j)
            for t in range(T):
                if t == 0:
                    for j in pair:
                        emit_u(j, 0)
                for j in pair:
                    emit_ubcv(j, t)
                for j in pair:
                    emit_echain(j, t)
                for j in pair:
                    emit_ds(j, t)
                for j in pair:
                    emit_stl(j, t)
                for j in pair:
                    emit_ctx(j, t)
                for j in pair:
                    emit_gru(j, t)
                if t == 0:
                    for j in nxt:
                        emit_g0(j)
                    for j in nxt:
                        emit_w01(j)
    nc.finalize()
    return nc, ctx


def _prep_core(node_feats_bf, seg, g_lo, g_hi, NT, NSUB):
    """Stage one core's node data in the four device layouts."""
    NN = NSUB * 128
    nfaug = np.zeros((128, NT, NSUB, F + 1), NP_BF16)
    nft = np.zeros((128, NT, 2, NN), NP_BF16)
    mn = np.zeros((128, NT, 128, NSUB), NP_BF16)
    mnt = np.zeros((128, NT, NSUB, 128), NP_BF16)
    eye = np.eye(128, dtype=NP_BF16)
    for j in range(NT):
        gt = g_lo + j * 128
        if gt >= g_hi:
            continue
        ge = min(gt + 128, g_hi)
        a = int(np.searchsorted(seg, gt, 'left'))
        b = int(np.searchsorted(seg, ge, 'left'))
        cnt = b - a
        assert cnt <= NN
        tmp = np.zeros((NN, F + 1), NP_BF16)
        tmp[:cnt, 0] = 1.0
        tmp[:cnt, 1:] = node_feats_bf[a:b]
        # node n -> subtile s=n//128, partition p=n%128
        nfaug[:, j] = tmp.reshape(NSUB, 128, F + 1).transpose(1, 0, 2)
        # nft[fp, j, c, n] = nf[a+n, c*128+fp]
        nft[:, j] = np.ascontiguousarray(
            tmp[:, 1:].T.reshape(2, 128, NN).transpose(1, 0, 2))
        grel = np.full(NN, -1, np.int64)
        grel[:cnt] = seg[a:b] - gt
        oh = eye[np.clip(grel, 0, 127)] * (grel >= 0)[:, None].astype(NP_BF16)
        oh = oh.reshape(NSUB, 128, 128)          # [s, p, g]
        mn[:, j] = oh.transpose(1, 2, 0)         # [p, g, s]
        mnt[:, j] = oh.transpose(2, 0, 1)        # [g, s, p]
    return nfaug, nft, mn, mnt


def kernel(node_feats, seg_ids, Wl, bl, Wp, bp, Wih, Whh, bih, bhh):
    node_feats = np.asarray(node_feats, np.float32)
    seg = np.asarray(seg_ids).astype(np.int64)
    Wl = np.asarray(Wl, np.float32)
    bl = np.asarray(bl, np.float32)
    Wp = np.asarray(Wp, np.float32)
    bp = np.asarray(bp, np.float32)
    Wih = np.asarray(Wih, np.float32)
    Whh = np.asarray(Whh, np.float32)
    bih = np.asarray(bih, np.float32)
    bhh = np.asarray(bhh, np.float32)
    V = node_feats.shape[0]
    G = 25000

    bounds_g = [0]
    for c in range(1, NCORES):
        bounds_g.append(int(seg[c * V // NCORES]))
    bounds_g.append(G)

    NT = max((bounds_g[c + 1] - bounds_g[c] + 127) // 128 for c in range(NCORES))
    maxnodes = 1
    for c in range(NCORES):
        for gt in range(bounds_g[c], bounds_g[c + 1], 128):
            ge = min(gt + 128, bounds_g[c + 1])
            a = np.searchsorted(seg, gt, 'left')
            b = np.searchsorted(seg, ge, 'left')
            maxnodes = max(maxnodes, int(b - a))
    NSUB = (maxnodes + 127) // 128

    nc, ctx = _build_program(NT, NSUB, [float(bl[t, 0]) for t in range(T)])

    shared = {
        "identb": np.eye(128, dtype=NP_BF16),
        "onesrow": np.ones((1, 128), NP_BF16),
    }
    wl2 = np.zeros((128, 2, T), np.float32)
    for t in range(T):
        for c in range(2):
            wl2[:, c, t] = Wl[t, 0, F + c * 128:F + (c + 1) * 128]
    shared["wl2"] = wl2.astype(NP_BF16)
    for t in range(T):
        shared[f"wlnf{t}"] = np.broadcast_to(Wl[t, 0, F:], (128, F)).astype(NP_BF16)
        shared[f"wlg{t}"] = np.broadcast_to(Wl[t, 0, :F], (128, F)).astype(NP_BF16)
        shared[f"wpt{t}"] = np.ascontiguousarray(
            Wp[t].T.reshape(2, 128, F).transpose(1, 0, 2)).astype(NP_BF16)
        shared[f"wih{t}"] = np.ascontiguousarray(
            Wih[t].T.reshape(2, 128, 3 * F).transpose(1, 0, 2)).astype(NP_BF16)
        shared[f"whh{t}"] = np.ascontiguousarray(
            Whh[t].T.reshape(2, 128, 3 * F).transpose(1, 0, 2)).astype(NP_BF16)
        shared[f"bprow{t}"] = bp[t][None, :].astype(NP_BF16)
        # elu's -1 shifted into the GRU input bias: x_gru = ctx+1 staged,
        # so bias_x -= rowsum(Wih)
        rs = Wih[t].sum(axis=1)
        shared[f"brzrow{t}"] = (bih[t, :2 * F] + bhh[t, :2 * F] - rs[:2 * F])[None, :].astype(NP_BF16)
        shared[f"binrow{t}"] = (bih[t, 2 * F:] - rs[2 * F:])[None, :].astype(NP_BF16)
        shared[f"bhnrow{t}"] = bhh[t, 2 * F:][None, :].astype(NP_BF16)

    node_feats_bf = node_feats.astype(NP_BF16)
    in_maps = []
    for c in range(NCORES):
        nfaug, nft, mn, mnt = _prep_core(
            node_feats_bf, seg, bounds_g[c], bounds_g[c + 1], NT, NSUB)
        m = dict(shared)
        m["nfaug"] = nfaug
        m["nft"] = nft
        m["mn"] = mn
        m["mnt"] = mnt
        in_maps.append(m)

    res = run_bass_kernel_spmd(nc, in_maps, core_ids=list(range(NCORES)))
    ctx.close()
    global LAST_RESULT
    LAST_RESULT = res

    out = np.zeros((G, F), np.float32)
    for c in range(NCORES):
        gc = bounds_g[c + 1] - bounds_g[c]
        out[bounds_g[c]:bounds_g[c + 1]] = res.results[c]["out"][:gc]
    return out
